# revision 7
# baseline (speedup 1.0000x reference)
"""CrossAttention (B=1, S=4096, H=8, DH=40) on 8 Trainium2 NeuronCores.

Sharding: tensor-parallel over the 8 heads — core h computes head h's full
attention plus its partial output projection; the host sums the 8 partials
and adds the bias.

v2 dataflow (vs v1: single AV accumulator with column-packed chunk pairs,
same-chunk AV lagged one exp-group behind the ScalarE stream, paired QK
from chunk 0, divide-free normalize split across the pair boundary):
  qT/kT  packed [104, 4096] images (q@0/k@64 and k@0/q@64) so the two
         ST matmuls of a j-group run on disjoint PE row groups
  ST     [128j, 512i] pairs -> exp on ScalarE [128, 1024] (the wall:
         ~16.7M exps at 1 elem/cycle/lane)
  v'     [128, 41] tiles (40 v cols + ones col 40) PE-transposed once
  AV     K=128 accumulation, even chunk at PE cols 0-40, odd chunk at
         cols 64-104 (tile_position=(0,64)) -> one PSUM bank per pair
  norm   r rows 40/104 -> SBUF-SBUF DMA to partition 0 -> reciprocal ->
         K=1 ones-matmul broadcast (cols 0 / 64) -> mult into oT_lo/oT_hi
  Y      [128, 320] = oT.T @ Wo_h.T, even tiles from rows 0-39, odd from
         rows 64-103 (duplicate woT image at partitions 64-103)
"""

import os

import ml_dtypes
import numpy as np

import concourse.bass as bass
import concourse.mybir as mybir
from concourse import bass_utils, masks
from concourse.tile import TileContext

S = 4096
D = 320
H = 8
DH = 40
N_CORES = 8
CHUNK = 512               # i-chunk width (one fp32 PSUM bank)
VW = 41                   # v' stationary width: 40 v cols, ones col 40
GJ = 2                    # j-tiles per exp group (2 PSUM banks)
SCALE = float(DH) ** -0.5
QKW = 104

F32 = mybir.dt.float32
BF16 = mybir.dt.bfloat16
EXP = mybir.ActivationFunctionType.Exp
MULT = mybir.AluOpType.mult

_COMPILED = {}


def _split_sync_waits(nc, max_waits=1):
    """This walrus build rejects instructions with more than one sync wait.
    Spill the excess onto same-engine nops placed just before the
    instruction (engine streams execute in program order, so all waits are
    satisfied before the instruction issues)."""
    for f in nc.m.functions:
        for bb in f.blocks:
            out = []
            changed = False
            for inst in bb.instructions:
                si = inst.sync_info
                if si is not None and si.on_wait and len(si.on_wait) > max_waits:
                    waits = list(si.on_wait)
                    for i in range(max_waits, len(waits), max_waits):
                        nop = mybir.InstNoOp(
                            name=nc.get_next_instruction_name(),
                            engine=inst.engine,
                            bass_nofuse=True,
                            sync_info=mybir.SyncInfo(
                                on_wait=waits[i:i + max_waits], on_update=[]),
                        )
                        out.append(nop)
                    inst.sync_info = mybir.SyncInfo(
                        on_wait=waits[:max_waits],
                        on_update=list(si.on_update or []))
                    changed = True
                out.append(inst)
            if changed:
                bb.instructions = out


def _build(s=None, split=True):
    s = s or S
    n_chunks = s // CHUNK
    jt = s // 128
    gpc = jt // GJ            # exp groups per chunk
    nc = bass.Bass('TRN2', target_bir_lowering=False, debug=False)

    xT_d = nc.dram_tensor('xT', [D, s], BF16, kind='ExternalInput').ap()
    wq_d = nc.dram_tensor('wq', [D, DH], BF16, kind='ExternalInput').ap()
    wk_d = nc.dram_tensor('wk', [D, DH], BF16, kind='ExternalInput').ap()
    wv_d = nc.dram_tensor('wv', [D, DH], BF16, kind='ExternalInput').ap()
    woT_d = nc.dram_tensor('woT', [DH, D], BF16, kind='ExternalInput').ap()
    out_d = nc.dram_tensor('out', [s, D], F32, kind='ExternalOutput').ap()

    KCH = (128, 128, 64)  # K chunks of D=320

    with TileContext(nc) as tc:
        with tc.tile_pool(name='const', bufs=1) as cpool, \
             tc.tile_pool(name='big', bufs=1) as big, \
             tc.tile_pool(name='pt', bufs=20) as ptp, \
             tc.tile_pool(name='work', bufs=3) as wkp, \
             tc.tile_pool(name='ps_st', bufs=2, space='PSUM') as ps_st, \
             tc.tile_pool(name='ps_small', bufs=2, space='PSUM') as ps_small, \
             tc.tile_pool(name='ps_av', bufs=1, space='PSUM') as ps_av, \
             tc.tile_pool(name='ps_warm', bufs=1, space='PSUM') as ps_warm:

            # ---- constants & inputs ----
            ident = cpool.tile([128, 128], F32, tag='ident')
            masks.make_identity(nc, ident[:, :])

            wA = cpool.tile([128, 3 * QKW], BF16, tag='wA')
            wB = cpool.tile([128, 3 * QKW], BF16, tag='wB')
            nc.vector.memset(wA[:, :], 0.0)
            nc.vector.memset(wB[:, :], 0.0)
            wv_sb = cpool.tile([128, 3 * DH], BF16, tag='wv')
            for c, kk in enumerate(KCH):
                o = sum(KCH[:c])
                nc.sync.dma_start(wA[0:kk, c * QKW:c * QKW + DH],
                                  wq_d[o:o + kk, :])
                nc.sync.dma_start(wA[0:kk, c * QKW + 64:c * QKW + QKW],
                                  wk_d[o:o + kk, :])
                nc.sync.dma_start(wB[0:kk, c * QKW:c * QKW + DH],
                                  wk_d[o:o + kk, :])
                nc.sync.dma_start(wB[0:kk, c * QKW + 64:c * QKW + QKW],
                                  wq_d[o:o + kk, :])
                nc.sync.dma_start(wv_sb[0:kk, c * DH:(c + 1) * DH],
                                  wv_d[o:o + kk, :])
            woT_sb = cpool.tile([DH, D], BF16, tag='woT')
            nc.sync.dma_start(woT_sb[:, :], woT_d)
            woT_hi = cpool.tile([QKW, D], BF16, tag='woT_hi')
            nc.sync.dma_start(woT_hi[64:64 + DH, :], woT_d)
            ones1 = cpool.tile([1, DH], F32, tag='ones1')
            nc.vector.memset(ones1[0:1, :], 1.0)

            xt0 = big.tile([128, s], BF16, tag='xt0')
            xt1 = big.tile([128, s], BF16, tag='xt1')
            xt2 = big.tile([64, s], BF16, tag='xt2')
            xts = (xt0, xt1, xt2)
            for c in range(n_chunks):
                cs = slice(c * CHUNK, (c + 1) * CHUNK)
                nc.sync.dma_start(xt0[:, cs], xT_d[0:128, cs])
                nc.sync.dma_start(xt1[:, cs], xT_d[128:256, cs])
                nc.sync.dma_start(xt2[:, cs], xT_d[256:320, cs])

            qkA = big.tile([QKW, s], BF16, tag='qkA')
            qkB = big.tile([QKW, s], BF16, tag='qkB')
            vT = big.tile([VW, s], F32, tag='vT')
            vsb = big.tile([128, jt * VW], BF16, tag='vsb')
            oT_lo = big.tile([DH, s], BF16, tag='oT_lo')
            oT_hi = big.tile([QKW, s], BF16, tag='oT_hi')

            # base-32 memset (DVE base must be 32-aligned); proj then
            # overwrites rows 32..39 with v, leaving row 40 = 1.0
            nc.vector.memset(vT[32:VW, :], 1.0)

            # ---- helpers ----
            def proj(dst, w_sb, c, ww):
                ps = ps_small.tile([QKW, CHUNK], F32, tag='small')
                for ci, kk in enumerate(KCH):
                    nc.tensor.matmul(
                        ps[0:ww, :],
                        w_sb[0:kk, ci * ww:(ci + 1) * ww],
                        xts[ci][0:kk, c * CHUNK:(c + 1) * CHUNK],
                        start=(ci == 0), stop=(ci == 2))
                nc.vector.tensor_copy(dst[:, c * CHUNK:(c + 1) * CHUNK],
                                      ps[0:ww, :])

            def transpose_v(j):
                tp = ps_small.tile([128, VW], F32, tag='small')
                nc.tensor.transpose(tp[:, 0:VW], vT[:, j * 128:(j + 1) * 128],
                                    ident[0:VW, 0:VW])
                nc.vector.tensor_copy(vsb[:, j * VW:(j + 1) * VW],
                                      tp[:, 0:VW])

            def warm(n=2):
                # keep-warm matmuls into the spare PSUM bank: the HAM
                # clock-gate halves the PE clock after idle windows, and a
                # ScalarE-paced steady state leaves the PE micro-idling
                # every slot. Junk full-array matmuls fill the gaps.
                w = ps_warm.tile([128, CHUNK], F32, tag='warm', name='warm')
                for _ in range(n):
                    nc.tensor.matmul(w[0:QKW, :], wA[:, 0:QKW],
                                     xt0[:, 0:CHUNK], start=True, stop=True)

            av_box = [None]

            def av_mm(c, pt, g):
                # two j-tile matmuls of group g of chunk c into the pair
                # accumulator: even chunk at PE cols 0-40, odd at 64-104
                lo = (c % 2 == 0)
                for k in range(GJ):
                    j = GJ * g + k
                    if j == 0 and lo:
                        av_box[0] = ps_av.tile([128, CHUNK], F32, tag='av',
                                               name='av')
                    av = av_box[0]
                    if lo:
                        nc.tensor.matmul(
                            av[0:VW, :], vsb[:, j * VW:(j + 1) * VW],
                            pt[:, k * CHUNK:(k + 1) * CHUNK],
                            start=(j == 0), stop=(j == jt - 1))
                    else:
                        nc.tensor.matmul(
                            av[64:64 + VW, :], vsb[:, j * VW:(j + 1) * VW],
                            pt[:, k * CHUNK:(k + 1) * CHUNK],
                            start=(j == 0), stop=(j == jt - 1),
                            tile_position=(0, 64))

            pair_state = {}

            def normalize_a(p):
                # drain the pair accumulator; move r rows to partition 0
                av = av_box[0]
                m = wkp.tile([105, CHUNK], F32, tag='m')
                nc.vector.tensor_copy(m[:, :], av[0:105, :])
                r0 = wkp.tile([1, CHUNK], F32, tag='r0')
                r1 = wkp.tile([1, CHUNK], F32, tag='r1')
                nc.sync.dma_start(r0[0:1, :], m[DH:DH + 1, :])
                nc.sync.dma_start(r1[0:1, :], m[104:105, :])
                rec0 = wkp.tile([1, CHUNK], F32, tag='rec0')
                rec1 = wkp.tile([1, CHUNK], F32, tag='rec1')
                nc.vector.reciprocal(rec0[0:1, :], r0[0:1, :])
                nc.vector.reciprocal(rec1[0:1, :], r1[0:1, :])
                pair_state[p] = (m, rec0, rec1)

            def normalize_b(p):
                # broadcast 1/r over partitions and scale into oT images
                m, rec0, rec1 = pair_state.pop(p)
                c0s = slice(2 * p * CHUNK, (2 * p + 1) * CHUNK)
                c1s = slice((2 * p + 1) * CHUNK, (2 * p + 2) * CHUNK)
                rbc0 = ps_small.tile([DH, CHUNK], F32, tag='small')
                nc.tensor.matmul(rbc0[:, :], ones1[0:1, :], rec0[0:1, :],
                                 start=True, stop=True)
                rb0 = wkp.tile([DH, CHUNK], F32, tag='rb')
                nc.vector.tensor_copy(rb0[:, :], rbc0[:, :])
                nc.vector.tensor_tensor(
                    out=oT_lo[:, c0s], in0=m[0:DH, :], in1=rb0[:, :], op=MULT)
                rbc1 = ps_small.tile([128, CHUNK], F32, tag='small')
                nc.tensor.matmul(rbc1[64:64 + DH, :], ones1[0:1, :],
                                 rec1[0:1, :], start=True, stop=True,
                                 tile_position=(0, 64))
                rb1 = wkp.tile([128, CHUNK], F32, tag='rb')
                nc.vector.tensor_copy(rb1[64:64 + DH, :], rbc1[64:64 + DH, :])
                nc.vector.tensor_tensor(
                    out=oT_hi[64:64 + DH, c1s], in0=m[64:64 + DH, :],
                    in1=rb1[64:64 + DH, :], op=MULT)

            def outproj_tile(st_i, hi):
                yp = ps_small.tile([128, D], F32, tag='small')
                if hi:
                    nc.tensor.matmul(yp[:, :],
                                     oT_hi[64:64 + DH,
                                           st_i * 128:(st_i + 1) * 128],
                                     woT_hi[64:64 + DH, :],
                                     start=True, stop=True)
                else:
                    nc.tensor.matmul(yp[:, :],
                                     oT_lo[:, st_i * 128:(st_i + 1) * 128],
                                     woT_sb[:, :], start=True, stop=True)
                ysb = wkp.tile([128, D], F32, tag='ysb')
                nc.vector.tensor_copy(ysb[:, :], yp[:, :])
                nc.sync.dma_start(out_d[st_i * 128:(st_i + 1) * 128, :],
                                  ysb[:, :])

            # ---- projections preamble (DMA-paced) ----
            for c in range(n_chunks):
                proj(qkB, wB, c, QKW)
                proj(qkA, wA, c, QKW)

            # ---- main loop over i-chunks ----
            tpc = CHUNK // 128  # s-tiles per chunk
            pts_c0 = None
            for c in range(n_chunks):
                pts = []
                cs = slice(c * CHUNK, (c + 1) * CHUNK)
                p_prev = (c - 2) // 2 if (c >= 2 and c % 2 == 0) else None
                for g in range(gpc):
                    st = ps_st.tile([128, GJ * CHUNK], F32, tag='st2')
                    j0, j1 = GJ * g, GJ * g + 1
                    nc.tensor.matmul(
                        st[:, 0:CHUNK],
                        qkB[0:DH, j0 * 128:(j0 + 1) * 128], qkA[0:DH, cs],
                        start=True, stop=True)
                    nc.tensor.matmul(
                        st[:, CHUNK:2 * CHUNK],
                        qkA[64:QKW, j1 * 128:(j1 + 1) * 128],
                        qkB[64:QKW, cs],
                        start=True, stop=True)
                    pt = ptp.tile([128, GJ * CHUNK], BF16, tag='pt')
                    nc.scalar.activation(pt[:, :], st[:, :], EXP, scale=SCALE)
                    pts.append(pt)

                    if c == 0:
                        # v projection + PE transposes hide in chunk 0
                        if g < n_chunks:
                            proj(vT[0:DH, :], wv_sb, g, DH)
                        transpose_v(2 * g)
                        transpose_v(2 * g + 1)
                    else:
                        if c == 1:
                            av_mm(0, pts_c0[g], g)
                        if g >= 1:
                            av_mm(c, pts[g - 1], g - 1)
                        warm(1 if c == 1 else 2)

                    if p_prev is not None:
                        if g == 0:
                            normalize_a(p_prev)
                        elif g == 9:
                            normalize_b(p_prev)
                        elif 11 <= g < 11 + tpc:
                            t = g - 11
                            outproj_tile(2 * p_prev * tpc + t, False)
                            outproj_tile((2 * p_prev + 1) * tpc + t, True)

                if c >= 1:
                    av_mm(c, pts[gpc - 1], gpc - 1)
                if c == 0:
                    pts_c0 = pts

            # ---- tail: last pair ----
            p = n_chunks // 2 - 1
            normalize_a(p)
            normalize_b(p)
            for t in range(tpc):
                outproj_tile(2 * p * tpc + t, False)
                outproj_tile((2 * p + 1) * tpc + t, True)

    if split:
        _split_sync_waits(nc)
    return nc


def kernel(x, Wq, Wk, Wv, Wo, bo):
    x = np.asarray(x, dtype=np.float32)
    Wq = np.asarray(Wq, dtype=np.float32)
    Wk = np.asarray(Wk, dtype=np.float32)
    Wv = np.asarray(Wv, dtype=np.float32)
    Wo = np.asarray(Wo, dtype=np.float32)
    bo = np.asarray(bo, dtype=np.float32)

    if 'nc' not in _COMPILED:
        _COMPILED['nc'] = _build()
    nc = _COMPILED['nc']

    bf = ml_dtypes.bfloat16
    xT = np.ascontiguousarray(x.reshape(S, D).T).astype(bf)
    in_maps = []
    for h in range(N_CORES):
        sl = slice(h * DH, (h + 1) * DH)
        in_maps.append({
            'xT': xT,
            'wq': np.ascontiguousarray(Wq[sl, :].T).astype(bf),
            'wk': np.ascontiguousarray(Wk[sl, :].T).astype(bf),
            'wv': np.ascontiguousarray(Wv[sl, :].T).astype(bf),
            'woT': np.ascontiguousarray(Wo[:, sl].T).astype(bf),
        })

    trace = bool(os.environ.get('BASS_KERNEL_TRACE'))

    def _run():
        return bass_utils.run_bass_kernel_spmd(
            nc, in_maps, core_ids=list(range(N_CORES)), trace=trace,
            tmpdir=os.environ.get('BASS_KERNEL_TRACE_DIR') or None)

    try:
        res = _run()
    except Exception:
        # A previously crashed NEFF can leave the device unrecoverable; the
        # failed attempt clears it, so one retry is usually enough.
        res = _run()
    _COMPILED['last_res'] = res

    acc = res.results[0]['out'].astype(np.float32).copy()
    for h in range(1, N_CORES):
        acc += res.results[h]['out']
    acc += bo[None, :]
    return acc.reshape(1, S, D)


# revision 8
# speedup vs baseline: 1.0441x; 1.0441x over previous
"""CrossAttention (B=1, S=4096, H=8, DH=40) on 8 Trainium2 NeuronCores.

Sharding: tensor-parallel over the 8 heads — core h computes head h's full
attention plus its partial output projection; the host sums the 8 partials
and adds the bias.

v2 dataflow (vs v1: single AV accumulator with column-packed chunk pairs,
same-chunk AV lagged one exp-group behind the ScalarE stream, paired QK
from chunk 0, divide-free normalize split across the pair boundary):
  qT/kT  packed [104, 4096] images (q@0/k@64 and k@0/q@64) so the two
         ST matmuls of a j-group run on disjoint PE row groups
  ST     [128j, 512i] pairs -> exp on ScalarE [128, 1024] (the wall:
         ~16.7M exps at 1 elem/cycle/lane)
  v'     [128, 41] tiles (40 v cols + ones col 40) PE-transposed once
  AV     K=128 accumulation, even chunk at PE cols 0-40, odd chunk at
         cols 64-104 (tile_position=(0,64)) -> one PSUM bank per pair
  norm   r rows 40/104 -> SBUF-SBUF DMA to partition 0 -> reciprocal ->
         K=1 ones-matmul broadcast (cols 0 / 64) -> mult into oT_lo/oT_hi
  Y      [128, 320] = oT.T @ Wo_h.T, even tiles from rows 0-39, odd from
         rows 64-103 (duplicate woT image at partitions 64-103)
"""

import os

import ml_dtypes
import numpy as np

import concourse.bass as bass
import concourse.mybir as mybir
from concourse import bass_utils, masks
from concourse.tile import TileContext

S = 4096
D = 320
H = 8
DH = 40
N_CORES = 8
CHUNK = 512               # i-chunk width (one fp32 PSUM bank)
VW = 41                   # v' stationary width: 40 v cols, ones col 40
GJ = 2                    # j-tiles per exp group (2 PSUM banks)
SCALE = float(DH) ** -0.5
QKW = 104

F32 = mybir.dt.float32
BF16 = mybir.dt.bfloat16
EXP = mybir.ActivationFunctionType.Exp
MULT = mybir.AluOpType.mult

_COMPILED = {}


def _split_sync_waits(nc, max_waits=1):
    """This walrus build rejects instructions with more than one sync wait.
    Spill the excess onto same-engine nops placed just before the
    instruction (engine streams execute in program order, so all waits are
    satisfied before the instruction issues)."""
    for f in nc.m.functions:
        for bb in f.blocks:
            out = []
            changed = False
            for inst in bb.instructions:
                si = inst.sync_info
                if si is not None and si.on_wait and len(si.on_wait) > max_waits:
                    waits = list(si.on_wait)
                    for i in range(max_waits, len(waits), max_waits):
                        nop = mybir.InstNoOp(
                            name=nc.get_next_instruction_name(),
                            engine=inst.engine,
                            bass_nofuse=True,
                            sync_info=mybir.SyncInfo(
                                on_wait=waits[i:i + max_waits], on_update=[]),
                        )
                        out.append(nop)
                    inst.sync_info = mybir.SyncInfo(
                        on_wait=waits[:max_waits],
                        on_update=list(si.on_update or []))
                    changed = True
                out.append(inst)
            if changed:
                bb.instructions = out


def _build(s=None, split=True):
    s = s or S
    n_chunks = s // CHUNK
    jt = s // 128
    gpc = jt // GJ            # exp groups per chunk
    nc = bass.Bass('TRN2', target_bir_lowering=False, debug=False)

    xT_d = nc.dram_tensor('xT', [D, s], BF16, kind='ExternalInput').ap()
    wq_d = nc.dram_tensor('wq', [D, DH], BF16, kind='ExternalInput').ap()
    wk_d = nc.dram_tensor('wk', [D, DH], BF16, kind='ExternalInput').ap()
    wv_d = nc.dram_tensor('wv', [D, DH], BF16, kind='ExternalInput').ap()
    woT_d = nc.dram_tensor('woT', [DH, D], BF16, kind='ExternalInput').ap()
    out_d = nc.dram_tensor('out', [s, D], F32, kind='ExternalOutput').ap()

    KCH = (128, 128, 64)  # K chunks of D=320

    with TileContext(nc) as tc:
        with tc.tile_pool(name='const', bufs=1) as cpool, \
             tc.tile_pool(name='big', bufs=1) as big, \
             tc.tile_pool(name='pt', bufs=20) as ptp, \
             tc.tile_pool(name='work', bufs=3) as wkp, \
             tc.tile_pool(name='ps_st', bufs=2, space='PSUM') as ps_st, \
             tc.tile_pool(name='ps_small', bufs=2, space='PSUM') as ps_small, \
             tc.tile_pool(name='ps_av', bufs=1, space='PSUM') as ps_av, \
             tc.tile_pool(name='ps_warm', bufs=1, space='PSUM') as ps_warm:

            # ---- constants & inputs ----
            ident = cpool.tile([128, 128], F32, tag='ident')
            masks.make_identity(nc, ident[:, :])

            wA = cpool.tile([128, 3 * QKW], BF16, tag='wA')
            wB = cpool.tile([128, 3 * QKW], BF16, tag='wB')
            nc.vector.memset(wA[:, :], 0.0)
            nc.vector.memset(wB[:, :], 0.0)
            wv_sb = cpool.tile([128, 3 * DH], BF16, tag='wv')
            for c, kk in enumerate(KCH):
                o = sum(KCH[:c])
                nc.sync.dma_start(wA[0:kk, c * QKW:c * QKW + DH],
                                  wq_d[o:o + kk, :])
                nc.sync.dma_start(wA[0:kk, c * QKW + 64:c * QKW + QKW],
                                  wk_d[o:o + kk, :])
                nc.sync.dma_start(wB[0:kk, c * QKW:c * QKW + DH],
                                  wk_d[o:o + kk, :])
                nc.sync.dma_start(wB[0:kk, c * QKW + 64:c * QKW + QKW],
                                  wq_d[o:o + kk, :])
                nc.sync.dma_start(wv_sb[0:kk, c * DH:(c + 1) * DH],
                                  wv_d[o:o + kk, :])
            woT_sb = cpool.tile([DH, D], BF16, tag='woT')
            nc.sync.dma_start(woT_sb[:, :], woT_d)
            woT_hi = cpool.tile([QKW, D], BF16, tag='woT_hi')
            nc.sync.dma_start(woT_hi[64:64 + DH, :], woT_d)
            ones1 = cpool.tile([1, DH], F32, tag='ones1')
            nc.vector.memset(ones1[0:1, :], 1.0)

            xt0 = big.tile([128, s], BF16, tag='xt0')
            xt1 = big.tile([128, s], BF16, tag='xt1')
            xt2 = big.tile([64, s], BF16, tag='xt2')
            xts = (xt0, xt1, xt2)
            for c in range(n_chunks):
                cs = slice(c * CHUNK, (c + 1) * CHUNK)
                nc.sync.dma_start(xt0[:, cs], xT_d[0:128, cs])
                nc.sync.dma_start(xt1[:, cs], xT_d[128:256, cs])
                nc.sync.dma_start(xt2[:, cs], xT_d[256:320, cs])

            qkA = big.tile([QKW, s], BF16, tag='qkA')
            qkB = big.tile([QKW, s], BF16, tag='qkB')
            vT = big.tile([VW, s], F32, tag='vT')
            vsb = big.tile([128, jt * VW], BF16, tag='vsb')
            oT_lo = big.tile([DH, s], BF16, tag='oT_lo')
            oT_hi = big.tile([QKW, s], BF16, tag='oT_hi')

            # base-32 memset (DVE base must be 32-aligned); proj then
            # overwrites rows 32..39 with v, leaving row 40 = 1.0
            nc.vector.memset(vT[32:VW, :], 1.0)

            # ---- helpers ----
            def proj(dst, w_sb, c, ww):
                ps = ps_small.tile([QKW, CHUNK], F32, tag='small')
                for ci, kk in enumerate(KCH):
                    nc.tensor.matmul(
                        ps[0:ww, :],
                        w_sb[0:kk, ci * ww:(ci + 1) * ww],
                        xts[ci][0:kk, c * CHUNK:(c + 1) * CHUNK],
                        start=(ci == 0), stop=(ci == 2))
                nc.vector.tensor_copy(dst[:, c * CHUNK:(c + 1) * CHUNK],
                                      ps[0:ww, :])

            def transpose_v(j):
                tp = ps_small.tile([128, VW], F32, tag='small')
                nc.tensor.transpose(tp[:, 0:VW], vT[:, j * 128:(j + 1) * 128],
                                    ident[0:VW, 0:VW])
                nc.vector.tensor_copy(vsb[:, j * VW:(j + 1) * VW],
                                      tp[:, 0:VW])

            def warm(n=2):
                # keep-warm matmuls into the spare PSUM bank: the HAM
                # clock-gate halves the PE clock after idle windows, and a
                # ScalarE-paced steady state leaves the PE micro-idling
                # every slot. Junk full-array matmuls fill the gaps.
                w = ps_warm.tile([128, CHUNK], F32, tag='warm', name='warm')
                for _ in range(n):
                    nc.tensor.matmul(w[0:QKW, :], wA[:, 0:QKW],
                                     xt0[:, 0:CHUNK], start=True, stop=True)

            av_box = [None]

            def av_mm(c, pt, g):
                # two j-tile matmuls of group g of chunk c into the pair
                # accumulator: even chunk at PE cols 0-40, odd at 64-104
                lo = (c % 2 == 0)
                for k in range(GJ):
                    j = GJ * g + k
                    if j == 0 and lo:
                        av_box[0] = ps_av.tile([128, CHUNK], F32, tag='av',
                                               name='av')
                    av = av_box[0]
                    if lo:
                        nc.tensor.matmul(
                            av[0:VW, :], vsb[:, j * VW:(j + 1) * VW],
                            pt[:, k * CHUNK:(k + 1) * CHUNK],
                            start=(j == 0), stop=(j == jt - 1))
                    else:
                        nc.tensor.matmul(
                            av[64:64 + VW, :], vsb[:, j * VW:(j + 1) * VW],
                            pt[:, k * CHUNK:(k + 1) * CHUNK],
                            start=(j == 0), stop=(j == jt - 1),
                            tile_position=(0, 64))

            pair_state = {}

            def normalize_a(p):
                # drain the pair accumulator; move r rows to partition 0
                av = av_box[0]
                m = wkp.tile([105, CHUNK], F32, tag='m')
                nc.vector.tensor_copy(m[:, :], av[0:105, :])
                r0 = wkp.tile([1, CHUNK], F32, tag='r0')
                r1 = wkp.tile([1, CHUNK], F32, tag='r1')
                nc.sync.dma_start(r0[0:1, :], m[DH:DH + 1, :])
                nc.sync.dma_start(r1[0:1, :], m[104:105, :])
                rec0 = wkp.tile([1, CHUNK], F32, tag='rec0')
                rec1 = wkp.tile([1, CHUNK], F32, tag='rec1')
                nc.vector.reciprocal(rec0[0:1, :], r0[0:1, :])
                nc.vector.reciprocal(rec1[0:1, :], r1[0:1, :])
                pair_state[p] = (m, rec0, rec1)

            def normalize_b(p):
                # broadcast 1/r over partitions and scale into oT images
                m, rec0, rec1 = pair_state.pop(p)
                c0s = slice(2 * p * CHUNK, (2 * p + 1) * CHUNK)
                c1s = slice((2 * p + 1) * CHUNK, (2 * p + 2) * CHUNK)
                rbc0 = ps_small.tile([DH, CHUNK], F32, tag='small')
                nc.tensor.matmul(rbc0[:, :], ones1[0:1, :], rec0[0:1, :],
                                 start=True, stop=True)
                rb0 = wkp.tile([DH, CHUNK], F32, tag='rb')
                nc.vector.tensor_copy(rb0[:, :], rbc0[:, :])
                nc.vector.tensor_tensor(
                    out=oT_lo[:, c0s], in0=m[0:DH, :], in1=rb0[:, :], op=MULT)
                rbc1 = ps_small.tile([128, CHUNK], F32, tag='small')
                nc.tensor.matmul(rbc1[64:64 + DH, :], ones1[0:1, :],
                                 rec1[0:1, :], start=True, stop=True,
                                 tile_position=(0, 64))
                rb1 = wkp.tile([128, CHUNK], F32, tag='rb')
                nc.vector.tensor_copy(rb1[64:64 + DH, :], rbc1[64:64 + DH, :])
                nc.vector.tensor_tensor(
                    out=oT_hi[64:64 + DH, c1s], in0=m[64:64 + DH, :],
                    in1=rb1[64:64 + DH, :], op=MULT)

            def outproj_tile(st_i, hi):
                yp = ps_small.tile([128, D], F32, tag='small')
                if hi:
                    nc.tensor.matmul(yp[:, :],
                                     oT_hi[64:64 + DH,
                                           st_i * 128:(st_i + 1) * 128],
                                     woT_hi[64:64 + DH, :],
                                     start=True, stop=True)
                else:
                    nc.tensor.matmul(yp[:, :],
                                     oT_lo[:, st_i * 128:(st_i + 1) * 128],
                                     woT_sb[:, :], start=True, stop=True)
                ysb = wkp.tile([128, D], F32, tag='ysb')
                nc.vector.tensor_copy(ysb[:, :], yp[:, :])
                nc.sync.dma_start(out_d[st_i * 128:(st_i + 1) * 128, :],
                                  ysb[:, :])

            # ---- projections preamble (DMA-paced) ----
            for c in range(n_chunks):
                proj(qkB, wB, c, QKW)
                proj(qkA, wA, c, QKW)

            # ---- main loop over i-chunks ----
            tpc = CHUNK // 128  # s-tiles per chunk
            pts_c0 = None
            for c in range(n_chunks):
                pts = []
                cs = slice(c * CHUNK, (c + 1) * CHUNK)
                p_prev = (c - 2) // 2 if (c >= 2 and c % 2 == 0) else None
                for g in range(gpc):
                    st = ps_st.tile([128, GJ * CHUNK], F32, tag='st2')
                    j0, j1 = GJ * g, GJ * g + 1
                    nc.tensor.matmul(
                        st[:, 0:CHUNK],
                        qkB[0:DH, j0 * 128:(j0 + 1) * 128], qkA[0:DH, cs],
                        start=True, stop=True)
                    nc.tensor.matmul(
                        st[:, CHUNK:2 * CHUNK],
                        qkA[64:QKW, j1 * 128:(j1 + 1) * 128],
                        qkB[64:QKW, cs],
                        start=True, stop=True)
                    pt = ptp.tile([128, GJ * CHUNK], BF16, tag='pt')
                    nc.scalar.activation(pt[:, :], st[:, :], EXP, scale=SCALE)
                    pts.append(pt)

                    if c == 0:
                        # v projection + PE transposes hide in chunk 0
                        if g < n_chunks:
                            proj(vT[0:DH, :], wv_sb, g, DH)
                        transpose_v(2 * g)
                        transpose_v(2 * g + 1)
                    else:
                        if c == 1:
                            av_mm(0, pts_c0[g], g)
                        if g >= 1:
                            av_mm(c, pts[g - 1], g - 1)

                    if p_prev is not None:
                        if g == 0:
                            normalize_a(p_prev)
                        elif g == 9:
                            normalize_b(p_prev)
                        elif 11 <= g < 11 + tpc:
                            t = g - 11
                            outproj_tile(2 * p_prev * tpc + t, False)
                            outproj_tile((2 * p_prev + 1) * tpc + t, True)

                if c >= 1:
                    av_mm(c, pts[gpc - 1], gpc - 1)
                if c == 0:
                    pts_c0 = pts

            # ---- tail: last pair ----
            p = n_chunks // 2 - 1
            normalize_a(p)
            normalize_b(p)
            for t in range(tpc):
                outproj_tile(2 * p * tpc + t, False)
                outproj_tile((2 * p + 1) * tpc + t, True)

    if split:
        _split_sync_waits(nc)
    return nc


def kernel(x, Wq, Wk, Wv, Wo, bo):
    x = np.asarray(x, dtype=np.float32)
    Wq = np.asarray(Wq, dtype=np.float32)
    Wk = np.asarray(Wk, dtype=np.float32)
    Wv = np.asarray(Wv, dtype=np.float32)
    Wo = np.asarray(Wo, dtype=np.float32)
    bo = np.asarray(bo, dtype=np.float32)

    if 'nc' not in _COMPILED:
        _COMPILED['nc'] = _build()
    nc = _COMPILED['nc']

    bf = ml_dtypes.bfloat16
    xT = np.ascontiguousarray(x.reshape(S, D).T).astype(bf)
    in_maps = []
    for h in range(N_CORES):
        sl = slice(h * DH, (h + 1) * DH)
        in_maps.append({
            'xT': xT,
            'wq': np.ascontiguousarray(Wq[sl, :].T).astype(bf),
            'wk': np.ascontiguousarray(Wk[sl, :].T).astype(bf),
            'wv': np.ascontiguousarray(Wv[sl, :].T).astype(bf),
            'woT': np.ascontiguousarray(Wo[:, sl].T).astype(bf),
        })

    trace = bool(os.environ.get('BASS_KERNEL_TRACE'))

    def _run():
        return bass_utils.run_bass_kernel_spmd(
            nc, in_maps, core_ids=list(range(N_CORES)), trace=trace,
            tmpdir=os.environ.get('BASS_KERNEL_TRACE_DIR') or None)

    try:
        res = _run()
    except Exception:
        # A previously crashed NEFF can leave the device unrecoverable; the
        # failed attempt clears it, so one retry is usually enough.
        res = _run()
    _COMPILED['last_res'] = res

    acc = res.results[0]['out'].astype(np.float32).copy()
    for h in range(1, N_CORES):
        acc += res.results[h]['out']
    acc += bo[None, :]
    return acc.reshape(1, S, D)
